# revision 1
# baseline (speedup 1.0000x reference)
"""Trainium2 Bass kernel for ContextQueryAttention (BiDAF-style).

Math (per batch):
    S[n,m] = c@w0 [n] + (q@w1 + bias)[m] + sum_d c[n,d]*wm[d]*q[m,d]
    S_  = softmax_m(S + MASK*(1-q_mask))          # row softmax
    S_T = softmax_n(S + MASK*(1-c_mask)).T        # col softmax, transposed
    c2q = S_ @ q ;  q2c = S_ @ (S_T @ c)
    out = [c | c2q | c*c2q | c*q2c]

Factorization used: with G = exp(sub2), A = exp(sub0), B = exp(sub1+bias),
the softmax ratios reduce to
    S_[n,m]  = G[n,m]*Bq[m] / (G @ Bq)[n]         Bq = B * q_mask
    S_T[m,n] = G[n,m]*Ac[n] / (G.T @ Ac)[m]       Ac = A * c_mask
so the big [N,M] matrix needs only one exp and no broadcast adds; A/B enter
as per-partition scalars on the small operands.  Denominators ride as an
extra column of the respective matmuls.  All contractions are fp32r
(full-rate PE, ~1e-4 rel err).  Sharding: data-parallel over batch, 8
batches per core on 8 cores.
"""

import sys

if "/opt/trn_rl_repo" not in sys.path:
    sys.path.insert(0, "/opt/trn_rl_repo")

import numpy as np

import concourse.bass as bass
import concourse.mybir as mybir
import concourse.tile as tile
from concourse import bacc
from concourse.bass_utils import run_bass_kernel_spmd
from concourse.masks import make_identity

B, N, M, D = 64, 1024, 128, 256
NCORES = 8
BPC = B // NCORES  # batches per core
NT = N // 128      # n-tiles per batch
DT = D // 128      # d-tiles

F32 = mybir.dt.float32
F32R = mybir.dt.float32r
I32 = mybir.dt.int32
EXP = mybir.ActivationFunctionType.Exp
MULT = mybir.AluOpType.mult
ADD = mybir.AluOpType.add


def _build(bpc: int = BPC, po_bufs: int = 4, big2_bufs: int = 2, tp_bufs: int = 2, big_bufs: int = 2):
    nc = bacc.Bacc(trn_type="TRN2")

    c_d = nc.dram_tensor("c", [bpc, N, D], F32, kind="ExternalInput")
    q_d = nc.dram_tensor("q", [bpc, M, D], F32, kind="ExternalInput")
    cm_d = nc.dram_tensor("c_mask", [bpc, N], I32, kind="ExternalInput")
    qm_d = nc.dram_tensor("q_mask", [bpc, M], I32, kind="ExternalInput")
    w0_d = nc.dram_tensor("w0", [D, 1], F32, kind="ExternalInput")
    w1_d = nc.dram_tensor("w1", [D, 1], F32, kind="ExternalInput")
    wm_d = nc.dram_tensor("wm", [D], F32, kind="ExternalInput")
    bias_d = nc.dram_tensor("bias", [M], F32, kind="ExternalInput")
    out_d = nc.dram_tensor("out", [bpc, N, 4 * D], F32, kind="ExternalOutput")

    with tile.TileContext(nc) as tc:
        with (
            tc.tile_pool(name="glob", bufs=1) as gp,
            tc.tile_pool(name="pb", bufs=2) as pb,
            tc.tile_pool(name="pscr", bufs=1) as pscr,
            tc.tile_pool(name="po", bufs=po_bufs) as po,
            tc.tile_pool(name="ps_tr", bufs=tp_bufs, space="PSUM") as ps_tr,
            tc.tile_pool(name="ps_big", bufs=big_bufs, space="PSUM") as ps_big,
            tc.tile_pool(name="ps_cq", bufs=big2_bufs, space="PSUM") as ps_cq,
        ):
            # ---- globals ----
            ident = gp.tile([128, 128], F32)
            make_identity(nc, ident)
            w0b = gp.tile([128, D], F32)
            nc.sync.dma_start(out=w0b, in_=w0_d[:, 0].partition_broadcast(128))
            w1b = gp.tile([128, D], F32)
            nc.sync.dma_start(out=w1b, in_=w1_d[:, 0].partition_broadcast(128))
            wm_sb = gp.tile([128, DT], F32)
            nc.sync.dma_start(out=wm_sb, in_=wm_d[:].rearrange("(j p) -> p j", p=128))
            bias_sb = gp.tile([128, 1], F32)
            nc.sync.dma_start(out=bias_sb, in_=bias_d[:].rearrange("(o p) -> p o", p=128))
            zeros8 = gp.tile([128, NT], F32)
            nc.vector.memset(zeros8, 0.0)

            def prep_stage(b):
                """Loads + everything up to tB for batch b."""
                st = {}
                c_n = pb.tile([128, NT, D], F32, tag="c_n")
                nc.sync.dma_start(
                    out=c_n, in_=c_d[b].rearrange("(i p) d -> p i d", p=128)
                )
                qb_t = pb.tile([128, D], F32, tag="qb_t")
                nc.sync.dma_start(out=qb_t, in_=q_d[b, :, :])
                qm_t = pb.tile([128, 1], I32, tag="qm_t")
                nc.sync.dma_start(
                    out=qm_t, in_=qm_d[b, :].rearrange("(o p) -> p o", p=128)
                )
                cm_t = pb.tile([128, NT], I32, tag="cm_t")
                nc.sync.dma_start(
                    out=cm_t, in_=cm_d[b, :].rearrange("(j p) -> p j", p=128)
                )
                mqf = pb.tile([128, 1], F32, tag="mqf")
                nc.vector.tensor_copy(mqf, qm_t)
                mcf = pb.tile([128, NT], F32, tag="mcf")
                nc.vector.tensor_copy(mcf, cm_t)

                # q-side prep
                scrq = pscr.tile([128, D], F32, tag="scrq")
                sub1 = pb.tile([128, 1], F32, tag="sub1")
                nc.vector.tensor_mul(scrq, qb_t, w1b)
                nc.vector.reduce_sum(out=sub1, in_=scrq, axis=mybir.AxisListType.X)
                bq0 = pb.tile([128, 1], F32, tag="bq0")
                nc.scalar.activation(bq0, sub1, EXP, bias=bias_sb, scale=1.0)
                bq = pb.tile([128, 1], F32, tag="bq")
                nc.vector.tensor_mul(bq, bq0, mqf)

                qwmT = pb.tile([128, DT, 128], F32R, tag="qwmT")
                tpq = ps_tr.tile([128, 256], F32, tag="tp")
                for j in range(DT):
                    nc.tensor.transpose(
                        tpq[:, 128 * j : 128 * (j + 1)],
                        qb_t[:, 128 * j : 128 * (j + 1)],
                        ident,
                    )
                    nc.vector.tensor_scalar_mul(
                        out=qwmT[:, j, :],
                        in0=tpq[:, 128 * j : 128 * (j + 1)],
                        scalar1=wm_sb[:, j : j + 1],
                    )
                qBx = pb.tile([128, D + 2], F32R, tag="qBx")
                nc.vector.tensor_scalar_mul(out=qBx[:, 0:D], in0=qb_t, scalar1=bq)
                nc.vector.tensor_copy(qBx[:, D : D + 2], zeros8[:, 0:2])
                nc.vector.tensor_copy(qBx[:, D : D + 1], bq)

                # cT via 4-wide PE transpose groups
                cT = pb.tile([128, DT, N], F32R, tag="cT")
                for ip in range(0, NT, 4):
                    for j in range(DT):
                        tp2 = ps_tr.tile([128, 512], F32, tag="tp")
                        for u in range(4):
                            nc.tensor.transpose(
                                tp2[:, 128 * u : 128 * (u + 1)],
                                c_n[:, ip + u, 128 * j : 128 * (j + 1)],
                                ident,
                            )
                        nc.scalar.copy(cT[:, j, 128 * ip : 128 * (ip + 4)], tp2)

                # ST matmul + exp -> GT
                GT = pb.tile([128, N], F32R, tag="GT")
                for h in range(2):
                    stp = ps_big.tile([128, 512], F32, tag="big")
                    for j in range(DT):
                        nc.tensor.matmul(
                            stp,
                            qwmT[:, j, :],
                            cT[:, j, 512 * h : 512 * (h + 1)],
                            start=(j == 0),
                            stop=(j == DT - 1),
                        )
                    nc.scalar.activation(GT[:, 512 * h : 512 * (h + 1)], stp, EXP)

                # G natural tiles (4-wide transpose groups)
                Gn = pb.tile([128, NT, 128], F32R, tag="Gn")
                for ip in range(0, NT, 4):
                    tp2 = ps_tr.tile([128, 512], F32, tag="tp")
                    for u in range(4):
                        nc.tensor.transpose(
                            tp2[:, 128 * u : 128 * (u + 1)],
                            GT[:, 128 * (ip + u) : 128 * (ip + u + 1)].bitcast(F32),
                            ident,
                        )
                    nc.scalar.copy(Gn[:, ip : ip + 4, :], tp2)

                # c-side per-tile scalars (batched)
                scrb = pscr.tile([128, NT, D], F32, tag="scrb")
                sub0 = pb.tile([128, NT], F32, tag="sub0")
                a0 = pb.tile([128, NT], F32, tag="a0")
                ac = pb.tile([128, NT], F32, tag="ac")
                cAx = pb.tile([128, NT, D + 2], F32R, tag="cAx")
                nc.vector.tensor_mul(
                    scrb, c_n, w0b.unsqueeze(1).to_broadcast([128, NT, D])
                )
                nc.vector.reduce_sum(out=sub0, in_=scrb, axis=mybir.AxisListType.X)
                nc.scalar.activation(a0, sub0, EXP)
                nc.vector.tensor_mul(ac, a0, mcf)
                for i in range(NT):
                    nc.vector.tensor_scalar_mul(
                        out=cAx[:, i, 0:D], in0=c_n[:, i, :], scalar1=ac[:, i : i + 1]
                    )
                nc.vector.tensor_copy(cAx[:, :, D + 1 : D + 2], zeros8.unsqueeze(2))
                nc.vector.tensor_copy(cAx[:, :, D : D + 1], ac.unsqueeze(2))

                # t = S_T @ c (numerator + cs column)
                tps = ps_big.tile([128, D + 2], F32, tag="big")
                for i in range(NT):
                    nc.tensor.matmul(
                        tps, Gn[:, i, :], cAx[:, i, :],
                        start=(i == 0), stop=(i == NT - 1),
                    )
                csi = pb.tile([128, 1], F32, tag="csi")
                nc.vector.reciprocal(csi, tps[:, D : D + 1])
                bqc = pb.tile([128, 1], F32, tag="bqc")
                nc.vector.tensor_mul(bqc, bq, csi)
                tB = pb.tile([128, D], F32R, tag="tB")
                nc.vector.tensor_scalar_mul(out=tB, in0=tps[:, 0:D], scalar1=bqc)

                st["c_n"] = c_n
                st["GT"] = GT
                st["qBx"] = qBx
                st["tB"] = tB
                return st

            def out_stage(b, st):
                """c2q/q2c matmuls, normalization, assembly, store for batch b."""
                c_n, GT, qBx, tB = st["c_n"], st["GT"], st["qBx"], st["tB"]
                rsi = pb.tile([128, NT], F32, tag="rsi")
                for i in range(NT):
                    gslice = GT[:, 128 * i : 128 * (i + 1)]
                    big2 = ps_cq.tile([128, 1024], F32, tag="big2")
                    nc.tensor.matmul(
                        big2[:, 0 : D + 2], gslice, qBx, start=True, stop=True
                    )
                    nc.tensor.matmul(
                        big2[:, 512 : 512 + D], gslice, tB, start=True, stop=True
                    )
                    nc.vector.reciprocal(rsi[:, i : i + 1], big2[:, D : D + 1])

                    # ot cols: [c | c2q | c*c2q | c*q2c]
                    ot = po.tile([128, 4 * D], F32, tag="ot")
                    nc.gpsimd.tensor_copy(ot[:, 0:D], c_n[:, i, :])
                    ot4 = ot.rearrange("p (j x) -> p j x", x=D)
                    ot_v = ot4[:, 1:4:2, :]
                    big_v = big2.rearrange("p (j x) -> p j x", j=2)[:, :, 0:D]
                    nc.scalar.mul(ot_v, big_v, rsi[:, i : i + 1])
                    nc.vector.tensor_mul(
                        ot[:, 2 * D : 4 * D],
                        ot_v,
                        c_n[:, i, :].unsqueeze(1).to_broadcast([128, 2, D]),
                    )
                    nc.scalar.dma_start(
                        out=out_d[b, 128 * i : 128 * (i + 1), :], in_=ot
                    )

            # software pipeline: prep(b+1) is emitted before out(b) so PE's
            # in-order stream overlaps consecutive batches
            prev = prep_stage(0)
            for b in range(bpc):
                nxt = prep_stage(b + 1) if b + 1 < bpc else None
                out_stage(b, prev)
                prev = nxt

    nc.finalize()
    return nc


_NC = None


def _get_nc():
    global _NC
    if _NC is None:
        _NC = _build()
    return _NC


def kernel(c, q, c_mask, q_mask, w0, w1, wm, bias):
    c = np.ascontiguousarray(c, dtype=np.float32)
    q = np.ascontiguousarray(q, dtype=np.float32)
    c_mask = np.ascontiguousarray(c_mask, dtype=np.int32)
    q_mask = np.ascontiguousarray(q_mask, dtype=np.int32)
    w0 = np.ascontiguousarray(w0, dtype=np.float32)
    w1 = np.ascontiguousarray(w1, dtype=np.float32)
    wm = np.ascontiguousarray(wm, dtype=np.float32)
    bias = np.ascontiguousarray(bias, dtype=np.float32)

    in_maps = []
    for k in range(NCORES):
        s = slice(k * BPC, (k + 1) * BPC)
        in_maps.append(
            {
                "c": c[s],
                "q": q[s],
                "c_mask": c_mask[s],
                "q_mask": q_mask[s],
                "w0": w0,
                "w1": w1,
                "wm": wm,
                "bias": bias,
            }
        )

    res = run_bass_kernel_spmd(_get_nc(), in_maps, core_ids=list(range(NCORES)))
    return np.concatenate([res.results[k]["out"] for k in range(NCORES)], axis=0)



# revision 14
# speedup vs baseline: 1.0038x; 1.0038x over previous
"""Trainium2 Bass kernel for ContextQueryAttention (BiDAF-style), v2.

Math (per batch):
    S[n,m] = c@w0 [n] + (q@w1 + bias)[m] + sum_d c[n,d]*wm[d]*q[m,d]
    S_  = softmax_m(S + MASK*(1-q_mask))          # row softmax
    S_T = softmax_n(S + MASK*(1-c_mask)).T        # col softmax, transposed
    c2q = S_ @ q ;  q2c = S_ @ (S_T @ c)
    out = [c | c2q | c*c2q | c*q2c]

Factorization: softmax ratios are invariant to per-row / per-column scalings,
so a single exp'd matrix serves both softmaxes:
    GTs[m,n] = exp(sub2[n,m] + sub1[m] + bias[m] + logqm[m])   # = G^T * Bq
    asn[n]   = exp(sub0[n] + logcm[n])
    S_[n,m]  = GTs[m,n] / den[n],   den[n] = sum_m GTs[m,n]
    S_T[m,n] = GTs[m,n]*asn[n] / cs[m]  up to a per-m factor that cancels
Masks are folded into the exponents with a soft value (-30): masked entries
contribute ~e^-30 relative weight (<< tolerance) and keep denominators
nonzero.  sub1 rides the exp as a per-partition activation bias; sub0 is a
[1,N] row matmul against the already-transposed cT; denominators ride the
matmuls as extra ones-columns.  G matrices are bf16 (range), streams are
fp16.  Sharding: data-parallel over batch, 8 batches per core on 8 cores.
"""

import sys

if "/opt/trn_rl_repo" not in sys.path:
    sys.path.insert(0, "/opt/trn_rl_repo")

import numpy as np

import concourse.bass as bass
import concourse.mybir as mybir
import concourse.tile as tile
from concourse import bacc
from concourse.bass_utils import run_bass_kernel_spmd
from concourse.masks import make_identity

B, N, M, D = 64, 1024, 128, 256
NCORES = 8
BPC = B // NCORES  # batches per core
NT = N // 128      # n-tiles per batch
DT = D // 128      # d-tiles
MASKV = -30.0  # soft mask: e^-30 relative leakage, keeps denoms nonzero
_STAGE = 99    # debug aid: emit only pipeline blocks <= _STAGE

F32 = mybir.dt.float32
F16 = mybir.dt.float16
BF16 = mybir.dt.bfloat16
EXP = mybir.ActivationFunctionType.Exp
MULT = mybir.AluOpType.mult
ADD = mybir.AluOpType.add


def _build(bpc: int = BPC):
    nc = bacc.Bacc(trn_type="TRN2")

    c_d = nc.dram_tensor("c", [bpc, N, D], BF16, kind="ExternalInput")
    q_d = nc.dram_tensor("q", [bpc, M, D], BF16, kind="ExternalInput")
    qwmT_d = nc.dram_tensor("qwmT", [bpc, D, M], BF16, kind="ExternalInput")
    w0c_d = nc.dram_tensor("w0c", [128, DT], BF16, kind="ExternalInput")
    w1_d = nc.dram_tensor("w1", [D], BF16, kind="ExternalInput")
    biasq_d = nc.dram_tensor("biasq", [128, bpc], F32, kind="ExternalInput")
    cmlog_d = nc.dram_tensor("cmlog", [128, bpc, NT], F32, kind="ExternalInput")
    out_d = nc.dram_tensor("out", [bpc, N, 4 * D], BF16, kind="ExternalOutput")

    with tile.TileContext(nc) as tc:
        with (
            tc.tile_pool(name="glob", bufs=1) as gp,
            tc.tile_pool(name="pb", bufs=2) as pb,
            tc.tile_pool(name="pscr", bufs=1) as pscr,
            tc.tile_pool(name="po", bufs=2) as po,
            tc.tile_pool(name="ps_tr", bufs=2, space="PSUM") as ps_tr,
            tc.tile_pool(name="ps_st", bufs=2, space="PSUM") as ps_st,
            tc.tile_pool(name="ps_out", bufs=2, space="PSUM") as ps_out,
        ):
            # ---- globals ----
            ident_b = gp.tile([128, 128], BF16)
            make_identity(nc, ident_b)
            w1b = gp.tile([128, D], BF16)
            nc.sync.dma_start(out=w1b, in_=w1_d[:].partition_broadcast(128))
            w0c = gp.tile([128, DT], BF16)
            nc.sync.dma_start(out=w0c, in_=w0c_d[:, :])
            biasq_sb = gp.tile([128, bpc], F32)
            nc.sync.dma_start(out=biasq_sb, in_=biasq_d[:, :])
            cmlog_sb = gp.tile([128, bpc, NT], F32)
            nc.sync.dma_start(out=cmlog_sb, in_=cmlog_d[:, :, :])

            def prep_stage(b):
                """Loads, cT, S^T matmul, GTs=exp, sub0->asn, Gn for batch b."""
                st = {}
                c_nx = pb.tile([128, NT, D + 2], BF16, tag="c_nx")
                nc.sync.dma_start(
                    out=c_nx[:, :, 0:D],
                    in_=c_d[b].rearrange("(i p) d -> p i d", p=128),
                )
                qt = pb.tile([128, 2 * D + 2], BF16, tag="qt")
                nc.sync.dma_start(out=qt[:, 0:D], in_=q_d[b, :, :])
                qwm = pb.tile([128, DT, M], BF16, tag="qwm")
                nc.sync.dma_start(
                    out=qwm, in_=qwmT_d[b].rearrange("(j p) m -> p j m", p=128)
                )
                if b < 2:  # const cols persist in the two pool buffers
                    nc.vector.memset(c_nx[:, :, D : D + 1], 1.0)
                    nc.vector.memset(c_nx[:, :, D + 1 : D + 2], 0.0)
                    nc.vector.memset(qt[:, D : D + 1], 1.0)
                    nc.vector.memset(qt[:, D + 1 : D + 2], 0.0)

                # sub1 + bias + logqm -> exp bias for GTs
                scr_q = pscr.tile([128, D], BF16, tag="scrq")
                sub1 = pb.tile([128, 1], F32, tag="sub1")
                bqb = pb.tile([128, 1], F32, tag="bqb")
                if _STAGE >= 1:
                    nc.vector.tensor_mul(scr_q, qt[:, 0:D], w1b)
                    nc.vector.reduce_sum(
                        out=sub1, in_=scr_q, axis=mybir.AxisListType.X
                    )
                    nc.vector.tensor_add(bqb, sub1, biasq_sb[:, b : b + 1])

                # cT via 4-wide PE transpose groups (fp16)
                cT = pb.tile([128, DT, N], BF16, tag="cT")
                for gi, ip in (enumerate((0, 4)) if _STAGE >= 2 else ()):
                    for j in range(DT):
                        tp = ps_tr.tile([128, 512], BF16, tag="tp")
                        for u in range(4):
                            nc.tensor.transpose(
                                tp[:, 128 * u : 128 * (u + 1)],
                                c_nx[:, ip + u, 128 * j : 128 * (j + 1)],
                                ident_b,
                            )
                        nc.vector.tensor_copy(cT[:, j, 128 * ip : 128 * (ip + 4)], tp)

                # S^T matmul (both 512-halves per stationary load) + exp
                GTs = pb.tile([128, N], BF16, tag="GTs")
                stp0 = ps_st.tile([128, 512], F32, tag="st")
                stp1 = ps_st.tile([128, 512], F32, tag="st")
                stp = [stp0, stp1]
                for j in (range(DT) if _STAGE >= 3 else ()):
                    for h in range(2):
                        nc.tensor.matmul(
                            stp[h],
                            qwm[:, j, :],
                            cT[:, j, 512 * h : 512 * (h + 1)],
                            start=(j == 0),
                            stop=(j == DT - 1),
                        )
                for h in (range(2) if _STAGE >= 3 else ()):
                    nc.scalar.activation(
                        GTs[:, 512 * h : 512 * (h + 1)], stp[h], EXP, bias=bqb
                    )

                # sub0 as [1,N] row matmul on cT, then slivers -> asn
                s0p0 = ps_st.tile([128, 512], F32, tag="st")
                s0p1 = ps_st.tile([128, 512], F32, tag="st")
                s0p = [s0p0, s0p1]
                for j in (range(DT) if _STAGE >= 4 else ()):
                    for h in range(2):
                        nc.tensor.matmul(
                            s0p[h][0:1, :],
                            w0c[:, j : j + 1],
                            cT[:, j, 512 * h : 512 * (h + 1)],
                            start=(j == 0),
                            stop=(j == DT - 1),
                        )
                s0sb = pb.tile([1, N], BF16, tag="s0sb")
                slv = ps_tr.tile([128, NT, 2], BF16, tag="tp")
                asn_f = pb.tile([128, NT], F32, tag="asnf")
                asn = pb.tile([128, NT], F32, tag="asn")
                if _STAGE >= 4:
                    for h in range(2):
                        nc.scalar.copy(
                            s0sb[0:1, 512 * h : 512 * (h + 1)], s0p[h][0:1, :]
                        )
                    for i in range(NT):
                        nc.tensor.transpose(
                            slv[:, i, 0:1],
                            s0sb[0:1, 128 * i : 128 * (i + 1)],
                            ident_b[0:1, 0:1],
                        )
                    nc.vector.tensor_add(asn_f, slv[:, :, 0], cmlog_sb[:, b, :])
                    nc.scalar.activation(asn, asn_f, EXP)

                # Gn tiles = transpose(GTs) * asn[n]  (bf16)
                Gn = pb.tile([128, NT, M], BF16, tag="Gn")
                for ip in ((0, 4) if _STAGE >= 5 else ()):
                    tpg = ps_tr.tile([128, 512], BF16, tag="tp")
                    for u in range(4):
                        nc.tensor.transpose(
                            tpg[:, 128 * u : 128 * (u + 1)],
                            GTs[:, 128 * (ip + u) : 128 * (ip + u + 1)],
                            ident_b,
                        )
                    for u in range(4):
                        nc.vector.tensor_scalar_mul(
                            out=Gn[:, ip + u, :],
                            in0=tpg[:, 128 * u : 128 * (u + 1)],
                            scalar1=asn[:, ip + u : ip + u + 1],
                        )

                st["c_nx"] = c_nx
                st["qt"] = qt
                st["GTs"] = GTs
                st["Gn"] = Gn
                return st

            def out_stage(b, st):
                """t matmul, c2q/q2c matmuls, normalize, assemble, store."""
                c_nx, qt, GTs, Gn = st["c_nx"], st["qt"], st["GTs"], st["Gn"]

                # t = (S_T-numerator @ [c|1|0]);  col D = cs[m]
                tps_buf = ps_st.tile([128, 512], F32, tag="st")
                tps = tps_buf[:, 0 : D + 2]
                csi = pb.tile([128, 1], F32, tag="csi")
                if _STAGE >= 6:
                    for i in range(NT):
                        nc.tensor.matmul(
                            tps, Gn[:, i, :], c_nx[:, i, :],
                            start=(i == 0), stop=(i == NT - 1),
                        )
                    nc.vector.reciprocal(csi, tps[:, D : D + 1])
                    nc.vector.tensor_scalar_mul(
                        out=qt[:, D + 2 : 2 * D + 2], in0=tps[:, 0:D], scalar1=csi
                    )

                cv = c_nx[:, :, 0:D]
                O2 = q3s = O3 = O4 = None
                if _STAGE >= 7:
                    O2 = po.tile([128, NT, D], BF16, tag="O2")
                    q3s = po.tile([128, NT, D], BF16, tag="q3s")
                if _STAGE >= 8:
                    O3 = po.tile([128, NT, D], BF16, tag="O3")
                    O4 = po.tile([128, NT, D], BF16, tag="O4")
                for i in (range(NT) if _STAGE >= 7 else ()):
                    gsl = GTs[:, 128 * i : 128 * (i + 1)]
                    pso = ps_out.tile([128, 768], F32, tag="big")
                    nc.tensor.matmul(
                        pso[:, 0 : D + 2], gsl, qt[:, 0 : D + 2],
                        start=True, stop=True,
                    )
                    nc.tensor.matmul(
                        pso[:, 512 : 512 + D], gsl, qt[:, D + 2 : 2 * D + 2],
                        start=True, stop=True,
                    )
                    rsi = pb.tile([128, 1], F32, tag="rsi")
                    nc.vector.reciprocal(rsi, pso[:, D : D + 1])
                    nc.vector.tensor_scalar_mul(
                        out=O2[:, i, :], in0=pso[:, 0:D], scalar1=rsi
                    )
                    nc.scalar.mul(q3s[:, i, :], pso[:, 512 : 512 + D], rsi)

                if _STAGE >= 8:
                    nc.gpsimd.tensor_mul(O3, O2, cv)
                    nc.gpsimd.tensor_mul(O4, q3s, cv)

                if _STAGE >= 8:
                    srcs = (cv, O2, O3, O4)
                elif _STAGE >= 7:
                    srcs = (cv, O2, O2, q3s)
                else:
                    srcs = (cv, cv, cv, cv)
                od = out_d[b].rearrange("(i p) e -> p i e", p=128)
                for qi, s_ in enumerate(srcs):
                    nc.sync.dma_start(out=od[:, :, qi * D : (qi + 1) * D], in_=s_)

            # software pipeline: prep(b+1) is emitted before out(b) so each
            # engine's in-order stream overlaps consecutive batches
            prev = prep_stage(0)
            for b in range(bpc):
                nxt = prep_stage(b + 1) if b + 1 < bpc else None
                out_stage(b, prev)
                prev = nxt

    nc.finalize()
    return nc


_NC = None


def _get_nc():
    global _NC
    if _NC is None:
        _NC = _build()
    return _NC


BF16NP = mybir.dt.np(BF16)


def _prep_inputs(c, q, c_mask, q_mask, w0, w1, wm, bias):
    """Host-side layout prep (casts, transposes, mask folding)."""
    c16 = np.ascontiguousarray(c, dtype=BF16NP)
    q16 = np.ascontiguousarray(q, dtype=BF16NP)
    qwmT = np.ascontiguousarray(
        (np.asarray(q, dtype=np.float32) * np.asarray(wm, dtype=np.float32)[None, None, :])
        .transpose(0, 2, 1),
        dtype=BF16NP,
    )  # [B, D, M]
    w0c = np.ascontiguousarray(
        np.asarray(w0, dtype=np.float32)[:, 0].reshape(DT, 128).T, dtype=BF16NP
    )  # [128, DT] p-major
    w116 = np.ascontiguousarray(np.asarray(w1, dtype=np.float32)[:, 0], dtype=BF16NP)
    biasq = (
        np.asarray(bias, dtype=np.float32)[None, :]
        + MASKV * (1.0 - np.asarray(q_mask, dtype=np.float32))
    )  # [B, M]
    cmlog = MASKV * (1.0 - np.asarray(c_mask, dtype=np.float32))  # [B, N]
    return c16, q16, qwmT, w0c, w116, biasq, cmlog


def build_in_maps(c, q, c_mask, q_mask, w0, w1, wm, bias):
    c16, q16, qwmT, w0c, w116, biasq, cmlog = _prep_inputs(
        c, q, c_mask, q_mask, w0, w1, wm, bias
    )

    in_maps = []
    for k in range(NCORES):
        s = slice(k * BPC, (k + 1) * BPC)
        in_maps.append(
            {
                "c": c16[s],
                "q": q16[s],
                "qwmT": qwmT[s],
                "w0c": w0c,
                "w1": w116,
                # [128, bpc] partition-major
                "biasq": np.ascontiguousarray(biasq[s].T),
                # [128, bpc, NT] partition-major
                "cmlog": np.ascontiguousarray(
                    cmlog[s].reshape(BPC, NT, 128).transpose(2, 0, 1)
                ),
            }
        )
    return in_maps


def kernel(c, q, c_mask, q_mask, w0, w1, wm, bias):
    in_maps = build_in_maps(c, q, c_mask, q_mask, w0, w1, wm, bias)
    res = run_bass_kernel_spmd(_get_nc(), in_maps, core_ids=list(range(NCORES)))
    out = np.concatenate([res.results[k]["out"] for k in range(NCORES)], axis=0)
    return np.asarray(out, dtype=np.float32)
